# revision 1
# baseline (speedup 1.0000x reference)
"""Trainium2 Bass kernel for nn_CrossAtt (cross-attention + concat + residual +
3x3 conv + BN + ReLU), data-parallel over (batch, row-group) across 8 cores.

Sharding: core i -> batch b = i//4, row-group rg = i%4 (16 output rows each).
Each core computes both attention branches for an 18-row extended window
(16 rows + 1 halo row each side, zero-masked at image edges), the fused
residual/concat outputs, and the 3x3 conv + BN + ReLU on its 16 rows.

All matmuls run as float32r (TF32-like, ~1.6e-4 rel err, full PE rate at
free-dim >= 256). fp32r operands must be produced by a compute op (DVE copy /
ACT) that rounds - DMA bitcasts crash the PE.
"""

import sys

sys.path.insert(0, "/opt/trn_rl_repo")

import numpy as np

import concourse.bacc as bacc
import concourse.tile as tile
from concourse import mybir
from concourse.bass_utils import run_bass_kernel_spmd

F32 = mybir.dt.float32
F32R = mybir.dt.float32r
AF = mybir.ActivationFunctionType
ALU = mybir.AluOpType

B, C, H, W = 2, 256, 64, 64
NW = H * W  # 4096 key/value positions
RE = 18  # extended rows per core
NE = RE * W  # 1152 query positions per core
D_QK, D_V = 16, 128
N_CORES = 8
BN_EPS = 1e-5
HALF = 576  # attention n-window half
CH = 288  # accumulator chunk

_PROG_CACHE: dict = {}


def _build_program(gamma: float):
    nc = bacc.Bacc("TRN2", target_bir_lowering=False, debug=False, num_devices=N_CORES)

    def din(name, shape):
        return nc.dram_tensor(name, shape, F32, kind="ExternalInput").ap()

    def dout(name, shape):
        return nc.dram_tensor(name, shape, F32, kind="ExternalOutput").ap()

    x1f = din("x1f", [C, NW])
    x2f = din("x2f", [C, NW])
    x1e = din("x1e", [C, NE])
    x2e = din("x2e", [C, NE])
    maskd = din("maskd", [128, NE])
    wqkd = din("wqkd", [128, 64])
    walld = din("walld", [128, 512])
    wcatd = din("wcatd", [128, 4608])
    bqkd = din("bqkd", [16, 2])
    bvd = din("bvd", [128, 1])
    bnd = din("bnd", [128, 4])
    o1 = dout("o1", [C, 1024])
    o2 = dout("o2", [C, 1024])
    feat = dout("feat", [C, 1024])

    xf = [x1f, x2f]
    xe = [x1e, x2e]

    with tile.TileContext(nc) as tc:
        with (
            tc.tile_pool(name="constp", bufs=1) as constp,
            tc.tile_pool(name="projp", bufs=1) as projp,
            tc.tile_pool(name="outp", bufs=1) as outp,
            tc.tile_pool(name="wcatp", bufs=1) as wcatp,
        ):
            # ---- constants: load + round to f32r ----
            with tc.tile_pool(name="ldtmp", bufs=2) as ldtmp:
                wqk_f = ldtmp.tile([128, 64], F32, name="wqk_f")
                nc.sync.dma_start(wqk_f[:], wqkd[:])
                wqk_r = constp.tile([128, 64], F32R, name="wqk_r")
                nc.vector.tensor_copy(wqk_r[:], wqk_f[:])

                wall_f = ldtmp.tile([128, 512], F32, name="wall_f")
                nc.sync.dma_start(wall_f[:], walld[:])
                wall_r = constp.tile([128, 512], F32R, name="wall_r")
                nc.vector.tensor_copy(wall_r[:], wall_f[:])

                ones_f = ldtmp.tile([128, 128], F32, name="ones_f")
                nc.gpsimd.memset(ones_f[:], 1.0)
                ones_r = constp.tile([128, 128], F32R, name="ones_r")
                nc.vector.tensor_copy(ones_r[:], ones_f[:])

            bqk_sb = constp.tile([16, 2], F32, name="bqk_sb")
            nc.sync.dma_start(bqk_sb[:], bqkd[:])
            bv_sb = constp.tile([128, 1], F32, name="bv_sb")
            nc.sync.dma_start(bv_sb[:], bvd[:])
            bn_sb = constp.tile([128, 4], F32, name="bn_sb")
            nc.sync.dma_start(bn_sb[:], bnd[:])
            mask_sb = constp.tile([128, NE], F32, name="mask_sb")
            nc.sync.dma_start(mask_sb[:], maskd[:])

            # extended-window inputs (residual path fp32 + rounded copy for q)
            xe_f = []
            for i in range(2):
                t = constp.tile([128, 2 * NE], F32, name=f"xe_f{i}")
                nc.sync.dma_start(t[:, 0:NE], xe[i][0:128, :])
                nc.sync.dma_start(t[:, NE : 2 * NE], xe[i][128:256, :])
                xe_f.append(t)

            # ---- projections ----
            k_r = [projp.tile([16, NW], F32R, name=f"k_r{i}") for i in range(2)]
            q_r = [projp.tile([16, NE], F32R, name=f"q_r{i}") for i in range(2)]
            vT = [projp.tile([128, NW], F32R, name=f"vT{i}") for i in range(2)]

            with (
                tc.tile_pool(name="xer", bufs=1) as xerp,
                tc.tile_pool(name="xsp", bufs=3) as xsp,
                tc.tile_pool(name="ps1", bufs=2, space="PSUM") as ps1,
            ):
                xe_r = []
                for i in range(2):
                    t = xerp.tile([128, 2 * NE], F32R, name=f"xe_r{i}")
                    nc.vector.tensor_copy(t[:], xe_f[i][:])
                    xe_r.append(t)

                for i in range(2):
                    for nt in range(8):
                        xt = xsp.tile([128, 1024], F32, name="xt")
                        c0 = nt * 512
                        # split issue across both HW-DGE queues (SP + ACT)
                        nc.sync.dma_start(xt[:, 0:512], xf[i][0:128, c0 : c0 + 512])
                        nc.scalar.dma_start(
                            xt[:, 512:1024], xf[i][128:256, c0 : c0 + 512]
                        )
                        xtr = xsp.tile([128, 1024], F32R, name="xtr")
                        nc.scalar.copy(xtr[:], xt[:])

                        # k of this input feeds the OTHER branch
                        psk = ps1.tile([16, 512], F32, name="psk")
                        for cc in range(2):
                            nc.tensor.matmul(
                                psk[:],
                                wqk_r[:, 32 + 16 * cc : 48 + 16 * cc],
                                xtr[:, 512 * cc : 512 * cc + 512],
                                start=(cc == 0),
                                stop=(cc == 1),
                            )
                        nc.vector.tensor_scalar_add(
                            k_r[1 - i][:, c0 : c0 + 512], psk[:], bqk_sb[:, 1:2]
                        )

                        # v^T via x-stationary matmul (out rows = positions)
                        for s4 in range(4):
                            psv = ps1.tile([128, 256], F32, name="psv")
                            for cc in range(2):
                                nc.tensor.matmul(
                                    psv[:],
                                    xtr[:, 512 * cc + 128 * s4 : 512 * cc + 128 * s4 + 128],
                                    wall_r[:, 256 * cc : 256 * cc + 256],
                                    start=(cc == 0),
                                    stop=(cc == 1),
                                )
                            m0 = (nt * 4 + s4) * 128
                            nc.vector.tensor_copy(
                                vT[i][:, m0 : m0 + 128], psv[:, 0:128]
                            )

                    # q projection from the extended window of input i
                    for j in range(3):
                        psq = ps1.tile([16, 384], F32, name="psk")
                        for cc in range(2):
                            nc.tensor.matmul(
                                psq[:],
                                wqk_r[:, 16 * cc : 16 * cc + 16],
                                xe_r[i][:, NE * cc + 384 * j : NE * cc + 384 * j + 384],
                                start=(cc == 0),
                                stop=(cc == 1),
                            )
                        nc.vector.tensor_scalar_add(
                            q_r[i][:, 384 * j : 384 * j + 384], psq[:], bqk_sb[:, 0:1]
                        )

            # ---- conv weights: load during attention (overlapped DMA) ----
            wcat_r = wcatp.tile([128, 4608], F32R, name="wcat_r")
            with tc.tile_pool(name="wctmp", bufs=2) as wctmp:
                for ch in range(2):
                    t = wctmp.tile([128, 2304], F32, name="wc_f")
                    nc.sync.dma_start(t[:], wcatd[:, 2304 * ch : 2304 * ch + 2304])
                    nc.vector.tensor_copy(
                        wcat_r[:, 2304 * ch : 2304 * ch + 2304], t[:]
                    )

            # ---- attention (both branches) ----
            out_e = [outp.tile([128, 2 * NE], F32, name=f"out_e{br}") for br in range(2)]

            with (
                tc.tile_pool(name="etp", bufs=4) as etp,
                tc.tile_pool(name="atp", bufs=2) as atp,
                tc.tile_pool(name="ps2", bufs=2, space="PSUM") as ps2,
            ):
                for br in range(2):
                    for half in range(2):
                        h0 = half * HALF
                        av = [
                            ps2.tile([128, CH], F32, name=f"av{j}", bufs=1)
                            for j in range(2)
                        ]
                        den = [
                            ps2.tile([1, CH], F32, name=f"den{j}", bufs=1)
                            for j in range(2)
                        ]
                        def acc_mm(mi, et):
                            # denominator + A@V accumulation for m-tile mi
                            for j in range(2):
                                ec = et[:, CH * j : CH * j + CH]
                                nc.tensor.matmul(
                                    den[j][:],
                                    ones_r[:, 0:1],
                                    ec,
                                    start=(mi == 0),
                                    stop=(mi == 31),
                                    skip_group_check=True,
                                )
                                nc.tensor.matmul(
                                    av[j][:],
                                    vT[br][:, mi * 128 : mi * 128 + 128],
                                    ec,
                                    start=(mi == 0),
                                    stop=(mi == 31),
                                    skip_group_check=True,
                                )

                        # software pipeline: S/exp of tile mi issue before the
                        # accumulators of tile mi-1, so PE fills the exp latency
                        from collections import deque

                        pend = deque()
                        for mi in range(32):
                            s_t = ps2.tile([128, HALF], F32, name="s_t")
                            nc.tensor.matmul(
                                s_t[:, 0:512],
                                k_r[br][:, mi * 128 : mi * 128 + 128],
                                q_r[br][:, h0 : h0 + 512],
                                start=True,
                                stop=True,
                            )
                            nc.tensor.matmul(
                                s_t[:, 512:HALF],
                                k_r[br][:, mi * 128 : mi * 128 + 128],
                                q_r[br][:, h0 + 512 : h0 + HALF],
                                start=True,
                                stop=True,
                            )
                            et = etp.tile([128, HALF], F32R, name="et")
                            nc.scalar.activation(et[:], s_t[:], AF.Exp, scale=0.25)
                            pend.append((mi, et))
                            if len(pend) > 2:
                                m, e = pend.popleft()
                                acc_mm(m, e)
                        while pend:
                            m, e = pend.popleft()
                            acc_mm(m, e)

                        recip_f = atp.tile([1, HALF], F32, name="recip_f")
                        for j in range(2):
                            nc.vector.reciprocal(
                                recip_f[:, CH * j : CH * j + CH], den[j][:]
                            )
                        recip_r = atp.tile([1, HALF], F32R, name="recip_r")
                        nc.vector.tensor_copy(recip_r[:], recip_f[:])

                        for j in range(2):
                            bb = ps2.tile([128, CH], F32, name="bb", tag="s_t")
                            nc.tensor.matmul(
                                bb[:],
                                ones_r[0:1, :],
                                recip_r[:, CH * j : CH * j + CH],
                                start=True,
                                stop=True,
                            )
                            bcp = atp.tile([128, CH], F32, name="bcp")
                            nc.scalar.copy(bcp[:], bb[:])
                            attn_t = atp.tile([128, CH], F32, name="attn_t")
                            nc.vector.tensor_mul(attn_t[:], av[j][:], bcp[:])
                            attn2 = atp.tile([128, CH], F32, name="attn2")
                            nc.vector.tensor_scalar_add(attn2[:], attn_t[:], bv_sb[:])
                            c0 = h0 + CH * j
                            nc.vector.scalar_tensor_tensor(
                                out_e[br][:, c0 : c0 + CH],
                                attn2[:],
                                gamma,
                                xe_f[br][:, c0 : c0 + CH],
                                ALU.mult,
                                ALU.add,
                            )
                            nc.vector.scalar_tensor_tensor(
                                out_e[br][:, NE + c0 : NE + c0 + CH],
                                xe_f[br][:, c0 : c0 + CH],
                                gamma,
                                xe_f[br][:, NE + c0 : NE + c0 + CH],
                                ALU.mult,
                                ALU.add,
                            )

            # ---- store branch outputs ----
            for br, od in ((0, o1), (1, o2)):
                nc.sync.dma_start(od[0:128, :], out_e[br][:, 64:1088])
                nc.sync.dma_start(od[128:256, :], out_e[br][:, NE + 64 : NE + 1088])

            # ---- s = mask*(out1+out2), zero-padded width; conv + BN + ReLU ----
            with (
                tc.tile_pool(name="convp", bufs=2) as convp,
                tc.tile_pool(name="spadp", bufs=1) as spadp,
                tc.tile_pool(name="ps3", bufs=2, space="PSUM") as ps3,
            ):
                zz = spadp.tile([128, RE], F32, name="zz")
                nc.gpsimd.memset(zz[:], 0.0)
                spad = []
                for cc in range(2):
                    sm = convp.tile([128, NE], F32, name="sm")
                    nc.vector.tensor_add(
                        sm[:],
                        out_e[0][:, NE * cc : NE * cc + NE],
                        out_e[1][:, NE * cc : NE * cc + NE],
                    )
                    sp = spadp.tile([128, RE, 66], F32R, name=f"spad{cc}")
                    nc.vector.tensor_copy(sp[:, :, 0:1], zz[:])
                    nc.vector.tensor_copy(sp[:, :, 65:66], zz[:])
                    nc.vector.tensor_mul(sp[:, :, 1:65], sm[:], mask_sb[:])
                    spad.append(sp)

                for oc in range(2):
                    for hh in range(2):
                        psy = ps3.tile([128, 512], F32, name="psy")
                        first = True
                        for t in range(9):
                            dy, dx = t // 3, t % 3
                            for cc in range(2):
                                nc.tensor.matmul(
                                    psy[:],
                                    wcat_r[
                                        :,
                                        2304 * cc
                                        + 256 * t
                                        + 128 * oc : 2304 * cc
                                        + 256 * t
                                        + 128 * oc
                                        + 128,
                                    ],
                                    spad[cc][:, 8 * hh + dy : 8 * hh + dy + 8, dx : dx + 64],
                                    start=first,
                                    stop=(t == 8 and cc == 1),
                                    skip_group_check=True,
                                )
                                first = False
                        fs = convp.tile([128, 512], F32, name="fs")
                        nc.scalar.activation(
                            fs[:],
                            psy[:],
                            AF.Relu,
                            bias=bn_sb[:, 2 + oc : 3 + oc],
                            scale=bn_sb[:, oc : oc + 1],
                        )
                        nc.sync.dma_start(
                            feat[128 * oc : 128 * oc + 128, 512 * hh : 512 * hh + 512],
                            fs[:],
                        )

    nc.compile()
    return nc


def _prep_inputs(input1, input2, Wq, bq, Wk, bk, Wv, bv, gamma, Wcat, bn_gamma, bn_beta):
    f32 = np.float32
    x1 = np.ascontiguousarray(np.asarray(input1, f32).reshape(B, C, NW))
    x2 = np.ascontiguousarray(np.asarray(input2, f32).reshape(B, C, NW))
    Wq, Wk, Wv = (np.asarray(w, f32) for w in (Wq, Wk, Wv))
    Wcat = np.asarray(Wcat, f32)

    wqk = np.zeros((128, 64), f32)
    wqk[:, 0:16] = Wq.T[0:128]
    wqk[:, 16:32] = Wq.T[128:256]
    wqk[:, 32:48] = Wk.T[0:128]
    wqk[:, 48:64] = Wk.T[128:256]

    wall = np.zeros((128, 512), f32)
    wall[:, 0:128] = Wv.T[0:128]
    wall[:, 256:384] = Wv.T[128:256]

    # [t, c, o] -> per c-chunk [128, 9*256]
    Wt = Wcat.transpose(2, 3, 1, 0).reshape(9, 256, 256)
    wcat2 = np.zeros((128, 4608), f32)
    for cc in range(2):
        wcat2[:, 2304 * cc : 2304 * (cc + 1)] = (
            Wt[:, 128 * cc : 128 * (cc + 1), :].transpose(1, 0, 2).reshape(128, 2304)
        )

    bqk = np.zeros((16, 2), f32)
    bqk[:, 0] = np.asarray(bq, f32)
    bqk[:, 1] = np.asarray(bk, f32)
    bvv = np.asarray(bv, f32).reshape(128, 1)

    bnscale = (np.asarray(bn_gamma, f32) / np.sqrt(f32(1.0) + f32(BN_EPS))).astype(f32)
    bnb = np.asarray(bn_beta, f32)
    bn = np.zeros((128, 4), f32)
    bn[:, 0] = bnscale[0:128]
    bn[:, 1] = bnscale[128:256]
    bn[:, 2] = bnb[0:128]
    bn[:, 3] = bnb[128:256]

    in_maps = []
    for core in range(N_CORES):
        b, rg = core // 4, core % 4
        r0 = 16 * rg
        lo = r0 - 1

        def ext(x):
            e = np.zeros((C, RE, W), f32)
            g0, g1 = max(0, lo), min(H, lo + RE)
            e[:, g0 - lo : g1 - lo, :] = x[b].reshape(C, H, W)[:, g0:g1, :]
            return e.reshape(C, NE)

        rows = np.ones(RE, f32)
        if rg == 0:
            rows[0] = 0.0
        if rg == 3:
            rows[RE - 1] = 0.0
        mask = np.broadcast_to(np.repeat(rows, W)[None, :], (128, NE)).copy()

        in_maps.append(
            {
                "x1f": np.ascontiguousarray(x1[b]),
                "x2f": np.ascontiguousarray(x2[b]),
                "x1e": ext(x1),
                "x2e": ext(x2),
                "maskd": mask,
                "wqkd": wqk,
                "walld": wall,
                "wcatd": wcat2,
                "bqkd": bqk,
                "bvd": bvv,
                "bnd": bn,
            }
        )
    return in_maps


def _assemble(results):
    f32 = np.float32
    feat_sum = np.empty((B, C, H, W), f32)
    out1 = np.empty((B, C, H, W), f32)
    out2 = np.empty((B, C, H, W), f32)
    for core in range(N_CORES):
        b, rg = core // 4, core % 4
        r0 = 16 * rg
        r = results[core]
        out1[b, :, r0 : r0 + 16] = r["o1"].reshape(C, 16, W)
        out2[b, :, r0 : r0 + 16] = r["o2"].reshape(C, 16, W)
        feat_sum[b, :, r0 : r0 + 16] = r["feat"].reshape(C, 16, W)
    return feat_sum, out1, out2


def _get_program(gamma: float):
    if gamma not in _PROG_CACHE:
        _PROG_CACHE[gamma] = _build_program(gamma)
    return _PROG_CACHE[gamma]


def kernel(input1, input2, Wq, bq, Wk, bk, Wv, bv, gamma, Wcat, bn_gamma, bn_beta):
    g = float(np.asarray(gamma).reshape(-1)[0])
    nc = _get_program(g)
    in_maps = _prep_inputs(
        input1, input2, Wq, bq, Wk, bk, Wv, bv, gamma, Wcat, bn_gamma, bn_beta
    )
    res = run_bass_kernel_spmd(nc, in_maps, core_ids=list(range(N_CORES)))
    return _assemble(res.results)


def run_traced(inputs):
    """For test.py: run with NTFF tracing, return (outputs, exec_time_ns)."""
    g = float(np.asarray(inputs["gamma"]).reshape(-1)[0])
    nc = _get_program(g)
    in_maps = _prep_inputs(**inputs)
    res = run_bass_kernel_spmd(nc, in_maps, core_ids=list(range(N_CORES)))
    return _assemble(res.results), res.exec_time_ns



# revision 8
# speedup vs baseline: 1.6881x; 1.6881x over previous
"""Trainium2 Bass kernel for nn_CrossAtt (dual cross-attention + concat +
residual + 3x3 conv + BN + ReLU), data-parallel over (batch, row-group)
across 8 cores.

Sharding: core i -> batch b = i//4, row-group rg = i%4 (16 output rows).
Host pre-ROLLS each core's x1/x2 along the flattened HW axis so the core's
18-row extended query window (16 rows + 1 halo row each side) sits at a
FIXED column range [0, 1152) of the rolled buffer. Attention is invariant
to the consistent key permutation the roll induces; edge-wrap halo rows are
zeroed by the conv mask, exactly like the out-of-image rows they replace.

Numerics / engine assignment:
 - x, Wq/Wk/Wv, Wcat in bf16 (full PE rate at any free size; halves DMA;
   kills all fp32->fp32r rounding copies).
 - S = k^T q accumulated in fp32 PSUM; exp on ACT with scale=1/4 and
   bias=-2 (range guard), output E in fp8e4.
 - A@V and softmax denominator via fp8 DoubleRow matmuls (two 128-key
   chunks per instruction).
 - Softmax normalization: DVE reciprocal + GPSIMD partition_broadcast
   (no ones-matmul / ACT copy).
 - conv runs as 2-row output slabs interleaved between attention blocks;
   BN+ReLU fused on ACT.
"""

import sys

sys.path.insert(0, "/opt/trn_rl_repo")

from collections import deque

import numpy as np
import ml_dtypes

import concourse.bacc as bacc
import concourse.tile as tile
from concourse import mybir
from concourse.bass_utils import run_bass_kernel_spmd

F32 = mybir.dt.float32
BF16 = mybir.dt.bfloat16
FP8 = mybir.dt.float8e4
AF = mybir.ActivationFunctionType
ALU = mybir.AluOpType
DR = mybir.MatmulPerfMode.DoubleRowSwInterleave

B, C, H, W = 2, 256, 64, 64
NW = H * W  # 4096 key positions
RE = 18  # extended rows per core (16 + halo)
NE = RE * W  # 1152 query positions
D_QK, D_V = 16, 128
N_CORES = 8
BN_EPS = 1e-5
BW = 384  # query block width (3 blocks per branch)
NBLK = NE // BW

_PROG_CACHE: dict = {}


def _build_program(gamma: float):
    nc = bacc.Bacc("TRN2", target_bir_lowering=False, debug=False, num_devices=N_CORES)

    def din(name, shape, dt=F32):
        return nc.dram_tensor(name, shape, dt, kind="ExternalInput").ap()

    def dout(name, shape):
        return nc.dram_tensor(name, shape, F32, kind="ExternalOutput").ap()

    x1d = din("x1d", [C, NW], BF16)
    x2d = din("x2d", [C, NW], BF16)
    wqkd = din("wqkd", [128, 2, 32], BF16)
    wvd = din("wvd", [128, 2, 128], BF16)
    wcatd = din("wcatd", [128, 2, 9, 256], BF16)
    maskd = din("maskd", [128, RE, W], BF16)
    bqkd = din("bqkd", [16, 2])
    bvgd = din("bvgd", [128, 1])
    bnd = din("bnd", [128, 4])
    o1 = dout("o1", [C, 1024])
    o2 = dout("o2", [C, 1024])
    feat = dout("feat", [C, 1024])

    with tile.TileContext(nc) as tc:
        with (
            tc.tile_pool(name="constp", bufs=1) as constp,
            tc.tile_pool(name="projp", bufs=1) as projp,
            tc.tile_pool(name="outp", bufs=1) as outp,
            tc.tile_pool(name="ep", bufs=4) as ep,
            tc.tile_pool(name="wkp", bufs=3) as wkp,
            tc.tile_pool(name="spool", bufs=2, space="PSUM") as spool,
            tc.tile_pool(name="app", bufs=1, space="PSUM") as app,
            tc.tile_pool(name="pjp", bufs=1, space="PSUM") as pjp,
        ):
            # ---- persistent SBUF tiles ----
            x1 = constp.tile([128, 2, NW], BF16, name="x1")
            x2 = constp.tile([128, 2, NW], BF16, name="x2")
            wqk = constp.tile([128, 2, 32], BF16, name="wqk")
            wv = constp.tile([128, 2, 128], BF16, name="wv")
            wcat = constp.tile([128, 2, 9, 256], BF16, name="wcat")
            mask = constp.tile([128, RE, W], BF16, name="mask")
            bqk = constp.tile([16, 2], F32, name="bqk")
            bvg = constp.tile([128, 1], F32, name="bvg")
            bn = constp.tile([128, 4], F32, name="bn")
            ones_dr = constp.tile([128, 2, 128], FP8, name="ones_dr")
            negc = constp.tile([128, 1], F32, name="negc")

            k_r = [projp.tile([16, NW], BF16, name=f"k_r{i}") for i in range(2)]
            q_r = [projp.tile([16, NE], BF16, name=f"q_r{i}") for i in range(2)]
            vT = [projp.tile([128, 16, 128, 2], FP8, name=f"vT{i}") for i in range(2)]
            xb = [projp.tile([128, NE], BF16, name=f"xb{i}") for i in range(2)]
            out_e = [outp.tile([128, 2, NE], F32, name=f"out_e{i}") for i in range(2)]
            spad = outp.tile([128, 2, RE, W + 2], BF16, name="spad")

            xs = [x1, x2]  # self input per branch
            xo = [x2, x1]  # other input per branch

            # ---- DMA issue: sync queue gets cc0, scalar queue gets cc1 ----
            nc.sync.dma_start(bqk[:], bqkd[:])
            nc.sync.dma_start(bvg[:], bvgd[:])
            nc.scalar.dma_start(wqk[:], wqkd[:])
            nc.scalar.dma_start(wv[:], wvd[:])
            # x quarters, x2 then x1 alternating so k-proj chase starts early
            QW = 1024
            for q in range(4):
                for xt, xd in ((x2, x2d), (x1, x1d)):
                    nc.sync.dma_start(
                        xt[:, 0, QW * q : QW * q + QW], xd[0:128, QW * q : QW * q + QW]
                    )
                    nc.scalar.dma_start(
                        xt[:, 1, QW * q : QW * q + QW],
                        xd[128:256, QW * q : QW * q + QW],
                    )
            nc.gpsimd.memset(ones_dr[:], 1.0)
            nc.gpsimd.memset(negc[:], -2.0)
            nc.gpsimd.memset(spad[:], 0.0)
            # late-needed tensors via the software DGE on the idle Pool engine
            nc.gpsimd.dma_start(mask[:], maskd[:])
            nc.gpsimd.dma_start(bn[:], bnd[:])
            nc.gpsimd.dma_start(wcat[:], wcatd[:])

            # ---- emission helpers ----
            def emit_psk(br, nt):
                """k projection for 512 keys: k of branch br comes from x_other."""
                psk = pjp.tile([16, 512], F32, name="psk")
                c0 = nt * 512
                for cc in range(2):
                    nc.tensor.matmul(
                        psk[:],
                        wqk[:, cc, 16:32],
                        xo[br][:, cc, c0 : c0 + 512],
                        start=(cc == 0),
                        stop=(cc == 1),
                    )
                nc.vector.tensor_scalar_add(k_r[br][:, c0 : c0 + 512], psk[:], bqk[:, 1:2])

            def emit_psv(br, nt):
                """v projection for 4 key chunks -> vT[br][:, 4nt:4nt+4, :] fp8."""
                psv = pjp.tile([128, 512], F32, name="psv")
                for s4 in range(4):
                    p0 = nt * 512 + s4 * 128
                    for cc in range(2):
                        nc.tensor.matmul(
                            psv[:, s4 * 128 : s4 * 128 + 128],
                            xs[br][:, cc, p0 : p0 + 128],
                            wv[:, cc, :],
                            start=(cc == 0),
                            stop=(cc == 1),
                            skip_group_check=True,
                        )
                for h in range(2):
                    for sub in range(2):
                        nc.vector.tensor_copy(
                            vT[br][:, 2 * nt + h : 2 * nt + h + 1, :, sub : sub + 1],
                            psv[:, (2 * h + sub) * 128 : (2 * h + sub + 1) * 128],
                        )

            def emit_psq(br, blk):
                psq = pjp.tile([16, BW], F32, name="psq", tag="psk")
                q0 = blk * BW
                for cc in range(2):
                    nc.tensor.matmul(
                        psq[:],
                        wqk[:, cc, 0:16],
                        xs[br][:, cc, q0 : q0 + BW],
                        start=(cc == 0),
                        stop=(cc == 1),
                    )
                nc.vector.tensor_scalar_add(q_r[br][:, q0 : q0 + BW], psq[:], bqk[:, 0:1])

            def emit_xb(br):
                # x_self low channels + gamma*bv (residual+bias base for concat half)
                nc.vector.tensor_scalar_add(xb[br][:], xs[br][:, 0, 0:NE], bvg[:])

            # conv slabs: slab k covers output window rows {2k+1, 2k+2}
            def emit_conv_slab(k, oc):
                psy = pjp.tile([128, 128], F32, name="psy", tag=("psk" if oc == 0 else "psv"))
                first = True
                for t in range(9):
                    dy, dx = t // 3, t % 3
                    for cc in range(2):
                        nc.tensor.matmul(
                            psy[:],
                            wcat[:, cc, t, oc * 128 : oc * 128 + 128],
                            spad[:, cc, 2 * k + dy : 2 * k + dy + 2, dx : dx + 64],
                            start=first,
                            stop=(t == 8 and cc == 1),
                            skip_group_check=True,
                        )
                        first = False
                fs = wkp.tile([128, 128], F32, name="fs")
                nc.scalar.activation(
                    fs[:], psy[:], AF.Relu, bias=bn[:, 2 + oc : 3 + oc], scale=bn[:, oc : oc + 1]
                )
                nc.sync.dma_start(
                    feat[128 * oc : 128 * oc + 128, 128 * k : 128 * k + 128], fs[:]
                )

            def emit_spad_prep(b):
                """spad rows 6b..6b+6 = mask * (out1+out2), bf16."""
                r0, q0 = 6 * b, 6 * b * W
                for cc in range(2):
                    ssum = wkp.tile([128, BW], F32, name="ssum")
                    nc.vector.tensor_add(
                        ssum[:], out_e[0][:, cc, q0 : q0 + BW], out_e[1][:, cc, q0 : q0 + BW]
                    )
                    nc.vector.tensor_tensor(
                        spad[:, cc, r0 : r0 + 6, 1:65],
                        ssum[:],
                        mask[:, r0 : r0 + 6, :],
                        ALU.mult,
                    )

            # ---- attention block ----
            def emit_block(br, blk, chase=None, between=None):
                """chase: dict pair_idx -> list of thunks emitted before that pair.
                between: list of thunks sprinkled after avden pops."""
                q0 = blk * BW
                av = app.tile([128, BW], F32, name="av")
                den = app.tile([128, BW], F32, name="den")
                pend = deque()
                between = list(between or [])

                def pop_one():
                    p, E = pend.popleft()
                    nc.tensor.matmul(
                        av[:],
                        vT[br][:, p : p + 1, :, :],
                        E[:],
                        start=(p == 0),
                        stop=(p == 15),
                        perf_mode=DR,
                        skip_group_check=True,
                    )
                    nc.tensor.matmul(
                        den[:],
                        ones_dr[:],
                        E[:],
                        start=(p == 0),
                        stop=(p == 15),
                        perf_mode=DR,
                        skip_group_check=True,
                    )

                for p in range(16):
                    if chase and p in chase:
                        for th in chase[p]:
                            th()
                    s_t = spool.tile([128, 2, 512], F32, name="s_t")
                    for j in range(2):
                        nc.tensor.matmul(
                            s_t[:, j, 0:BW],
                            k_r[br][:, (2 * p + j) * 128 : (2 * p + j) * 128 + 128],
                            q_r[br][:, q0 : q0 + BW],
                            start=True,
                            stop=True,
                        )
                    E = ep.tile([128, 2, BW], FP8, name="E")
                    nc.scalar.activation(E[:], s_t[:, :, 0:BW], AF.Exp, scale=0.25, bias=negc[:])
                    pend.append((p, E))
                    if len(pend) > 2:
                        pop_one()
                        if between:
                            between.pop(0)()
                while pend:
                    pop_one()
                for th in between:
                    th()

                # normalization + residual/concat epilogue
                rb = wkp.tile([128, BW], F32, name="rb")
                nc.vector.reciprocal(rb[:], den[:])
                tmp = wkp.tile([128, BW], F32, name="tmp")
                nc.vector.tensor_tensor(tmp[:], av[:], rb[:], ALU.mult)
                nc.vector.scalar_tensor_tensor(
                    out_e[br][:, 0, q0 : q0 + BW], tmp[:], gamma, xb[br][:, q0 : q0 + BW],
                    ALU.mult, ALU.add,
                )
                nc.vector.scalar_tensor_tensor(
                    out_e[br][:, 1, q0 : q0 + BW],
                    xs[br][:, 0, q0 : q0 + BW], gamma, xs[br][:, 1, q0 : q0 + BW],
                    ALU.mult, ALU.add,
                )

            # ---- emission schedule ----
            # block idx0 (br0, b0): chase br0 k/v projections + q tiles
            chase0 = {}
            for nt in range(8):
                ops = []
                if nt == 0:
                    ops += [lambda: emit_psk(0, 0), lambda: emit_psv(0, 0),
                            lambda: emit_psq(0, 0)]
                else:
                    ops += [lambda nt=nt: emit_psk(0, nt), lambda nt=nt: emit_psv(0, nt)]
                if nt == 1:
                    ops.append(lambda: emit_psq(1, 0))
                if nt == 4:
                    ops.append(lambda: emit_xb(0))
                if nt == 5:
                    ops.append(lambda: emit_xb(1))
                chase0[2 * nt] = ops
            emit_block(0, 0, chase=chase0)

            # block idx1 (br1, b0): chase br1 projections
            chase1 = {}
            for nt in range(8):
                ops = [lambda nt=nt: emit_psk(1, nt), lambda nt=nt: emit_psv(1, nt)]
                if nt == 2:
                    ops.append(lambda: emit_psq(0, 1))
                if nt == 3:
                    ops.append(lambda: emit_psq(1, 1))
                chase1[2 * nt] = ops
            emit_block(1, 0, chase=chase1)

            # conv rows for b0 are ready; slabs k0,k1 interleave into idx2/idx3
            emit_spad_prep(0)
            emit_block(0, 1, chase={2: [lambda: emit_psq(0, 2)]},
                       between=[lambda: emit_conv_slab(0, 0), lambda: emit_conv_slab(0, 1)])
            emit_block(1, 1, chase={2: [lambda: emit_psq(1, 2)]},
                       between=[lambda: emit_conv_slab(1, 0), lambda: emit_conv_slab(1, 1)])

            emit_spad_prep(1)
            emit_block(0, 2, between=[
                lambda: emit_conv_slab(2, 0), lambda: emit_conv_slab(2, 1),
                lambda: emit_conv_slab(3, 0), lambda: emit_conv_slab(3, 1),
            ])
            # br0 fully done: store o1
            def store_o1():
                nc.sync.dma_start(o1[0:128, :], out_e[0][:, 0, 64:1088])
                nc.sync.dma_start(o1[128:256, :], out_e[0][:, 1, 64:1088])

            emit_block(1, 2, between=[
                lambda: emit_conv_slab(4, 0), lambda: emit_conv_slab(4, 1),
                store_o1,
                lambda: emit_conv_slab(5, 0), lambda: emit_conv_slab(5, 1),
            ])
            nc.sync.dma_start(o2[0:128, :], out_e[1][:, 0, 64:1088])
            nc.sync.dma_start(o2[128:256, :], out_e[1][:, 1, 64:1088])

            emit_spad_prep(2)
            for k in (6, 7):
                for oc in range(2):
                    emit_conv_slab(k, oc)

    nc.compile()
    return nc


def _prep_inputs(input1, input2, Wq, bq, Wk, bk, Wv, bv, gamma, Wcat, bn_gamma, bn_beta):
    f32 = np.float32
    bf16 = ml_dtypes.bfloat16
    g = f32(np.asarray(gamma).reshape(-1)[0])
    x1 = np.asarray(input1, f32).reshape(B, C, NW)
    x2 = np.asarray(input2, f32).reshape(B, C, NW)
    Wq, Wk, Wv = (np.asarray(w, f32) for w in (Wq, Wk, Wv))
    Wcat = np.asarray(Wcat, f32)

    wqk = np.zeros((128, 2, 32), f32)
    wv2 = np.zeros((128, 2, 128), f32)
    for cc in range(2):
        wqk[:, cc, 0:16] = Wq.T[128 * cc : 128 * cc + 128]
        wqk[:, cc, 16:32] = Wk.T[128 * cc : 128 * cc + 128]
        # column-reversed for the DoubleRowSwInterleave weight layout
        wv2[:, cc, :] = Wv.T[128 * cc : 128 * cc + 128][:, ::-1]

    # [t, cin, cout]
    Wt = Wcat.transpose(2, 3, 1, 0).reshape(9, 256, 256)
    wcat2 = np.zeros((128, 2, 9, 256), f32)
    for cc in range(2):
        wcat2[:, cc] = Wt[:, 128 * cc : 128 * cc + 128, :].transpose(1, 0, 2)

    bqk = np.zeros((16, 2), f32)
    bqk[:, 0] = np.asarray(bq, f32)
    bqk[:, 1] = np.asarray(bk, f32)
    bvgv = (g * np.asarray(bv, f32)).reshape(128, 1)

    bnscale = (np.asarray(bn_gamma, f32) / np.sqrt(f32(1.0) + f32(BN_EPS))).astype(f32)
    bnb = np.asarray(bn_beta, f32)
    bnt = np.zeros((128, 4), f32)
    bnt[:, 0] = bnscale[0:128]
    bnt[:, 1] = bnscale[128:256]
    bnt[:, 2] = bnb[0:128]
    bnt[:, 3] = bnb[128:256]

    wqk_b = wqk.astype(bf16)
    wv_b = wv2.astype(bf16)
    wcat_b = wcat2.astype(bf16)

    in_maps = []
    for core in range(N_CORES):
        b, rg = core // 4, core % 4
        roll = (16 * rg - 1) * 64  # window col j = image pos (roll + j) mod NW

        rows = np.ones(RE, f32)
        if rg == 0:
            rows[0] = 0.0
        if rg == 3:
            rows[RE - 1] = 0.0
        msk = np.broadcast_to(
            np.repeat(rows, W).reshape(RE, W)[None], (128, RE, W)
        ).astype(bf16)

        in_maps.append(
            {
                "x1d": np.ascontiguousarray(np.roll(x1[b], -roll, axis=1)).astype(bf16),
                "x2d": np.ascontiguousarray(np.roll(x2[b], -roll, axis=1)).astype(bf16),
                "wqkd": wqk_b,
                "wvd": wv_b,
                "wcatd": wcat_b,
                "maskd": msk,
                "bqkd": bqk,
                "bvgd": bvgv,
                "bnd": bnt,
            }
        )
    return in_maps


def _assemble(results):
    f32 = np.float32
    feat_sum = np.empty((B, C, H, W), f32)
    out1 = np.empty((B, C, H, W), f32)
    out2 = np.empty((B, C, H, W), f32)
    for core in range(N_CORES):
        b, rg = core // 4, core % 4
        r0 = 16 * rg
        r = results[core]
        out1[b, :, r0 : r0 + 16] = r["o1"].reshape(C, 16, W)
        out2[b, :, r0 : r0 + 16] = r["o2"].reshape(C, 16, W)
        feat_sum[b, :, r0 : r0 + 16] = r["feat"].reshape(C, 16, W)
    return feat_sum, out1, out2


def _get_program(gamma: float):
    if gamma not in _PROG_CACHE:
        _PROG_CACHE[gamma] = _build_program(gamma)
    return _PROG_CACHE[gamma]


def kernel(input1, input2, Wq, bq, Wk, bk, Wv, bv, gamma, Wcat, bn_gamma, bn_beta):
    g = float(np.asarray(gamma).reshape(-1)[0])
    nc = _get_program(g)
    in_maps = _prep_inputs(
        input1, input2, Wq, bq, Wk, bk, Wv, bv, gamma, Wcat, bn_gamma, bn_beta
    )
    res = run_bass_kernel_spmd(nc, in_maps, core_ids=list(range(N_CORES)))
    return _assemble(res.results)


def run_traced(inputs):
    """For test.py: run and return (outputs, exec_time_ns)."""
    g = float(np.asarray(inputs["gamma"]).reshape(-1)[0])
    nc = _get_program(g)
    in_maps = _prep_inputs(**inputs)
    res = run_bass_kernel_spmd(nc, in_maps, core_ids=list(range(N_CORES)))
    return _assemble(res.results), res.exec_time_ns
